# revision 10
# baseline (speedup 1.0000x reference)
"""Two-layer GAT on 8 Trainium2 NeuronCores (Bass/Tile, SPMD).

Sharding: dst nodes split into 784 tiles of 128; core c owns the 98
contiguous tiles = nodes [c*12544, (c+1)*12544).  Edges (incl.
self-loops) are grouped by (dst tile, src bank) with bank = src>>15 so
all per-edge rows can be fetched with batched multi-index dma_gather
instructions (int16 bank-local indices), instead of per-chunk indirect
DMAs (994ns fixed SWDGE cost each) that bottlenecked the Pool engine.

Layer tables: h1ext [NPAD, 256] bf16 rows [h(128)|ssrc(4)|sdst(4)|pad]
(512B stride as required by dma_gather); per-edge src rows gathered with
elem_size=136.  Per-edge dst scores come from a per-core local score
table sco1 [12672, 128] built with one dma_scatter_add (shard-local
int16 row ids, junk rows absorb other cores' nodes), gathered with
elem_size=4.  Layer 2 mirrors this with a repadded [NPAD, 128] h2 table
and an epilogue-written local sdst2 table.  Segment max is dropped
(scores are O(1); softmax is shift-invariant).
"""
import os
import sys

sys.path.insert(0, "/opt/trn_rl_repo")

import numpy as np

N = 100000
IN_DIM = 128
HID = 32
HEADS = 4
OUT_DIM = 32
NEG_SLOPE = 0.2

NC = 8
P = 128
NPAD = 100352          # 784 tiles of 128
SHARD = NPAD // NC     # 12544
NS = SHARD // P        # 98 dst tiles per core
NT = NPAD // P         # 784
W1C = 136              # h1(128) | ssrc1(4) | sdst1(4)
R1 = 256               # h1ext row elems (512B stride)
R2 = 128               # h2tab / sco row elems (256B stride)
W2C = 36               # h2(32) | ssrc2(1) | sdst2(1) | pad(2)
B = 7                  # dst tiles per processing batch
NB = NS // B           # 14 batches per core
BANK = 32768
NBANK = (NPAD + BANK - 1) // BANK      # 4 (banks 32768,32768,32768,2048)
LROW = SHARD + P       # local score-table rows (128 junk rows)

_RUNNER = None
_RUNNER_KEY = None


def _ap(t, ap_dims, extra_offset=0):
    import concourse.bass as bass
    base = t[:]
    return bass.AP(base.tensor, base.offset + extra_offset, ap_dims)


def _plan(nbc):
    """Static slot layout from per-(tile, bank) chunk counts nbc [NS, 4]."""
    cs = nbc.sum(axis=1)                      # chunks per tile
    scol = np.zeros(NS + 1, np.int64)
    np.cumsum(cs, out=scol[1:])               # dl (tile-major) col starts
    tcs = int(scol[NS])
    batches = []
    gbase = 0
    for bb in range(NB):
        ss = range(bb * B, bb * B + B)
        banks = []
        for b in range(NBANK):
            nch = int(sum(nbc[s][b] for s in ss))
            banks.append(nch)
        # gpos for (s, b, k): gbase + sum(banks[:b]) + sum(nbc[s'][b] for s'<s in batch) + k
        tile_off = {}
        for b in range(NBANK):
            off = 0
            for s in ss:
                tile_off[(s, b)] = off
                off += int(nbc[s][b])
        nbchunks = sum(banks)
        posmap = {}
        for s in ss:
            j = 0
            for b in range(NBANK):
                bank_off = sum(banks[:b])
                for k in range(int(nbc[s][b])):
                    posmap[(s, j)] = sum(banks[:b]) + tile_off[(s, b)] + k
                    j += 1
        batches.append(dict(gbase=gbase, banks=banks, nbchunks=nbchunks,
                            posmap=posmap))
        gbase += nbchunks
    assert gbase == tcs
    return dict(cs=cs, scol=scol, tcs=tcs, batches=batches)


def _build_program(nbc, nb_run=NB, p0_groups=None):
    from concourse import bass, mybir, bacc
    import concourse.tile as tile
    from concourse.masks import make_identity

    f32 = mybir.dt.float32
    bf16 = mybir.dt.bfloat16
    i32 = mybir.dt.int32
    i16 = mybir.dt.int16
    AF = mybir.ActivationFunctionType
    ALU = mybir.AluOpType

    plan = _plan(nbc)
    cs, scol, tcs = plan["cs"], plan["scol"], plan["tcs"]
    csmax = int(cs.max())
    nbcmax = max(b["nbchunks"] for b in plan["batches"])
    bank_rows = [min(BANK, NPAD - b * BANK) for b in range(NBANK)]

    nc = bacc.Bacc("TRN2", target_bir_lowering=False, debug=False, num_devices=NC)

    xT = nc.dram_tensor("xT", [P, NPAD], bf16, kind="ExternalInput")
    W1cat = nc.dram_tensor("W1cat", [P, W1C], bf16, kind="ExternalInput")
    W2cat = nc.dram_tensor("W2cat", [P, W2C], bf16, kind="ExternalInput")
    gidx = nc.dram_tensor("gidx", [P, tcs * 16], i16, kind="ExternalInput")
    scatidx = nc.dram_tensor("scatidx", [P, NPAD // 16], i16, kind="ExternalInput")
    edloc = nc.dram_tensor("edloc", [P, tcs], f32, kind="ExternalInput")
    out2 = nc.dram_tensor("out2", [SHARD, OUT_DIM], f32, kind="ExternalOutput")
    h1ext = nc.dram_tensor("h1ext", [NPAD, R1], bf16)
    h2tab = nc.dram_tensor("h2tab", [NPAD, R2], bf16)
    sco1 = nc.dram_tensor("sco1", [LROW, R2], bf16)
    sco2 = nc.dram_tensor("sco2", [LROW, R2], bf16)

    MAXCH = 64                     # max 8192 idxs per gather instruction

    def _dg1(gp, out_ap, in_ap, idxs_ap, num_idxs, elem, stride_elems):
        _in = gp.lower_ap_dma(in_ap, for_custom_bir_dma=True)
        return gp.add_instruction(mybir.InstDMAGatherAnt(
            name=gp.bass.get_next_instruction_name(),
            ins=[*_in, gp.lower_ap(idxs_ap),
                 gp.lower_val_access(gp.to_reg(num_idxs))],
            outs=[gp.lower_ap(out_ap)],
            transpose=False, num_idxs=num_idxs, elem_size=elem,
            stride_bytes_256=(stride_elems * 2) // 256, gen_mode=0,
            single_packet=False, queue_num=0, sbuf_tokens_per_rank=0,
            sbuf_free_dim_per_rank=0, sbuf_free_dim_pad_per_rank=0,
            sbuf_byte_offset=0))

    def dgather(gp, out_tile, out_col0, in_ap, idx_tile, idx_col0, nchunks,
                elem, stride_elems):
        # gather nchunks*128 rows of `elem` elems into out_tile cols starting
        # at out_col0*elem, splitting into <=MAXCH-chunk instructions
        done = 0
        while done < nchunks:
            nch = min(MAXCH, nchunks - done)
            _dg1(gp,
                 _ap(out_tile, [out_tile[:].ap[0], [elem, nch], [1, elem]],
                     extra_offset=(out_col0 + done) * elem),
                 in_ap,
                 idx_tile[:, idx_col0 + done * 8:idx_col0 + (done + nch) * 8],
                 nch * P, elem, stride_elems)
            done += nch

    with tile.TileContext(nc) as tc:
        with (
            tc.tile_pool(name="consts", bufs=1) as consts,
            tc.tile_pool(name="sb", bufs=2) as sb,
            tc.tile_pool(name="mp", bufs=2) as mp,
            tc.tile_pool(name="gx", bufs=3) as gx,
            tc.tile_pool(name="gg", bufs=2) as gg,
            tc.tile_pool(name="ix", bufs=2) as ix,
            tc.tile_pool(name="ps", bufs=2, space="PSUM") as ps,
            tc.tile_pool(name="pst", bufs=2, space="PSUM") as pst,
            tc.tile_pool(name="psagg", bufs=2, space="PSUM") as psagg,
            tc.tile_pool(name="psh2", bufs=2, space="PSUM") as psh2,
            tc.tile_pool(name="dram", bufs=1, space="DRAM") as dram,
        ):
            ident = consts.tile([P, P], f32)
            make_identity(nc, ident[:])
            iota_i = consts.tile([P, P], i32)
            nc.gpsimd.iota(iota_i[:], pattern=[[1, P]], base=0, channel_multiplier=0)
            iota_f = consts.tile([P, P], f32)
            nc.vector.tensor_copy(out=iota_f[:], in_=iota_i[:])
            w1_t = consts.tile([P, W1C], bf16)
            nc.sync.dma_start(out=w1_t[:], in_=W1cat[:])
            w2_t = consts.tile([P, W2C], bf16)
            nc.sync.dma_start(out=w2_t[:], in_=W2cat[:])
            dl_t = consts.tile([P, tcs], f32)
            nc.sync.dma_start(out=dl_t[:], in_=edloc[:])
            sct = consts.tile([P, NPAD // 16], i16)
            nc.sync.dma_start(out=sct[:], in_=scatidx[:])
            scores_all = consts.tile([P, NT * 4], bf16)
            sdst2_all = consts.tile([P, NS], bf16)

            # zero-init local score table sco1 (scatter-add target); reuse
            # scores_all as the zero source before phase 0 overwrites it
            nc.vector.memset(scores_all[:], 0.0)
            for k in range(6):
                nc.sync.dma_start(
                    out=_ap(sco1, [[LROW * R2 // P, P], [1, 2112]],
                            extra_offset=k * 2112),
                    in_=scores_all[:, 0:2112])

            # ---------- phase 0: h1ext rows = [x@W1 | x@W1s | x@W1d], all nodes
            GRP = 8
            _ng = NT // GRP if p0_groups is None else p0_groups
            for g in range(_ng):
                xg = gx.tile([P, P * GRP], bf16, tag="xg")
                nc.sync.dma_start(out=xg[:], in_=xT[:, g * P * GRP:(g + 1) * P * GRP])
                s0 = gx.tile([P, GRP * W1C], bf16, tag="s0")
                for t in range(GRP):
                    p0 = ps.tile([P, W1C], f32, tag="p0")
                    nc.tensor.matmul(out=p0[:], lhsT=xg[:, t * P:(t + 1) * P],
                                     rhs=w1_t[:], start=True, stop=True)
                    nc.scalar.copy(out=s0[:, t * W1C:(t + 1) * W1C], in_=p0[:])
                # stash sdst1 (cols 132:136 of each row) for the scatter
                nc.vector.tensor_copy(
                    out=scores_all[:, g * GRP * 4:(g + 1) * GRP * 4],
                    in_=_ap(s0, [s0[:].ap[0], [W1C, GRP], [1, 4]],
                            extra_offset=132))
                nc.sync.dma_start(
                    out=_ap(h1ext, [[R1, P], [P * R1, GRP], [1, W1C]],
                            extra_offset=g * GRP * P * R1),
                    in_=s0[:])

            # scatter own nodes' sdst1 into the shard-local table
            # (split: <=6272 idxs per instruction)
            for k in range(16):
                g0 = k * (NT // 16)
                nidx = (NT // 16) * P
                nc.gpsimd.dma_scatter_add(
                    out_ap=_ap(sco1, [[R2, LROW], [1, 4]]),
                    in_ap=_ap(scores_all,
                              [scores_all[:].ap[0], [4, NT // 16], [1, 4]],
                              extra_offset=g0 * 4),
                    idxs_ap=sct[:, g0 * 8:(g0 + NT // 16) * 8],
                    num_idxs=nidx, num_idxs_reg=nidx,
                    elem_size=4, elem_step=R2, single_packet=False)

            h2sh = dram.tile([SHARD, W2C], bf16)
            h2full = dram.tile([NPAD, W2C], bf16)

            # ---------- layer 1 edge pass, B tiles per batch
            for bb in range(nb_run):
                bat = plan["batches"][bb]
                nbch = bat["nbchunks"]
                ni = nbch * P
                itf = ix.tile([P, nbcmax * 16], i16, tag="it")
                it = itf[:, :nbch * 16]
                nc.sync.dma_start(
                    out=it, in_=gidx[:, bat["gbase"] * 16:
                                     (bat["gbase"] + nbch) * 16])

                G = gg.tile([P, nbcmax * W1C], bf16, tag="G")
                coff = 0
                for b in range(NBANK):
                    nch = bat["banks"][b]
                    if nch == 0:
                        continue
                    dgather(nc.gpsimd, G, coff,
                            _ap(h1ext, [[R1, bank_rows[b]], [1, W1C]],
                                extra_offset=b * BANK * R1),
                            itf, coff * 8, nch, W1C, R1)
                    coff += nch
                Df = sb.tile([P, nbcmax * 4], bf16, tag="D")
                D = Df[:, :nbch * 4]
                dgather(nc.gpsimd, Df, 0,
                        _ap(sco1, [[R2, LROW], [1, 4]]),
                        itf, nbch * 8, nbch, 4, R2)

                # scores: S = ssrc[src] + sdst[dst]; leaky; exp back into G
                Sf = sb.tile([P, nbcmax * 4], bf16, tag="S")
                S = Sf[:, :nbch * 4]
                nc.vector.tensor_tensor(
                    out=S,
                    in0=_ap(G, [G[:].ap[0], [W1C, nbch], [1, 4]], extra_offset=128),
                    in1=D, op=ALU.add)
                S2f = sb.tile([P, nbcmax * 4], bf16, tag="S2")
                S2 = S2f[:, :nbch * 4]
                nc.vector.scalar_tensor_tensor(
                    out=S2, in0=S, scalar=NEG_SLOPE, in1=S,
                    op0=ALU.mult, op1=ALU.max)
                nc.scalar.activation(
                    _ap(G, [G[:].ap[0], [W1C, nbch], [1, 4]], extra_offset=128),
                    _ap(S2f, [S2f[:].ap[0], [4, nbch], [1, 4]]), AF.Exp)
                nc.vector.tensor_tensor(
                    out=_ap(G, [G[:].ap[0], [W1C, nbch], [32, 4], [1, 32]]),
                    in0=_ap(G, [G[:].ap[0], [W1C, nbch], [32, 4], [1, 32]]),
                    in1=_ap(G, [G[:].ap[0], [W1C, nbch], [1, 4], [0, 32]],
                            extra_offset=128),
                    op=ALU.mult)

                hb = sb.tile([P, B * P], f32, tag="hb")
                for t in range(B):
                    s = bb * B + t
                    ncs = int(cs[s])
                    M = mp.tile([P, csmax * P], bf16, tag="M")
                    nc.vector.tensor_tensor(
                        out=_ap(M, [M[:].ap[0], [P, ncs], [1, P]]),
                        in0=_ap(dl_t, [dl_t[:].ap[0], [1, ncs], [0, P]],
                                extra_offset=int(scol[s])),
                        in1=_ap(iota_f, [iota_f[:].ap[0], [0, ncs], [1, P]]),
                        op=ALU.is_equal)
                    agg = psagg.tile([P, 132], f32, tag="agg")
                    for j in range(ncs):
                        gp0 = bat["posmap"][(s, j)] * W1C
                        nc.tensor.matmul(out=agg[:], lhsT=M[:, j * P:(j + 1) * P],
                                         rhs=G[:, gp0:gp0 + 132],
                                         start=(j == 0), stop=(j == ncs - 1))
                    den = sb.tile([P, 4], f32, tag="den")
                    nc.vector.tensor_scalar(out=den[:], in0=agg[:, 128:132],
                                            scalar1=1e-30, scalar2=None,
                                            op0=ALU.max)
                    rden = sb.tile([P, 4], f32, tag="rden")
                    nc.vector.reciprocal(out=rden[:], in_=den[:])
                    nc.vector.tensor_tensor(
                        out=_ap(hb, [hb[:].ap[0], [32, 4], [1, 32]],
                                extra_offset=t * P),
                        in0=_ap(agg, [agg[:].ap[0], [32, 4], [1, 32]]),
                        in1=_ap(rden, [rden[:].ap[0], [1, 4], [0, 32]]),
                        op=ALU.mult)

                # batched elu: helu = max(h,0) + (exp(min(h,0)) - 1)
                ta = sb.tile([P, B * P], f32, tag="ta")
                nc.vector.tensor_scalar(out=ta[:], in0=hb[:], scalar1=0.0,
                                        scalar2=None, op0=ALU.min)
                tb = sb.tile([P, B * P], f32, tag="tb")
                nc.scalar.activation(tb[:], ta[:], AF.Exp)
                nc.vector.tensor_scalar(out=ta[:], in0=hb[:], scalar1=0.0,
                                        scalar2=None, op0=ALU.max)
                nc.vector.scalar_tensor_tensor(
                    out=hb[:], in0=tb[:], scalar=-1.0, in1=ta[:],
                    op0=ALU.add, op1=ALU.add)

                h2sb = sb.tile([P, B * W2C], bf16, tag="h2sb")
                for t in range(B):
                    hT = pst.tile([P, P], f32, tag="hT")
                    nc.tensor.transpose(out=hT[:], in_=hb[:, t * P:(t + 1) * P],
                                        identity=ident[:])
                    hTs = sb.tile([P, P], bf16, tag="hTs")
                    nc.scalar.copy(out=hTs[:], in_=hT[:])
                    h2p = psh2.tile([P, W2C], f32, tag="h2p")
                    nc.tensor.matmul(out=h2p[:], lhsT=hTs[:], rhs=w2_t[:],
                                     start=True, stop=True)
                    nc.scalar.copy(out=h2sb[:, t * W2C:(t + 1) * W2C], in_=h2p[:])
                nc.vector.tensor_copy(
                    out=sdst2_all[:, bb * B:(bb + 1) * B],
                    in_=_ap(h2sb, [h2sb[:].ap[0], [W2C, B]], extra_offset=33))
                nc.sync.dma_start(
                    out=_ap(h2sh, [[W2C, P], [P * W2C, B], [1, W2C]],
                            extra_offset=bb * B * P * W2C),
                    in_=h2sb[:])

            # ---------- AllGather + repad h2 table; write local sdst2 table
            nc.gpsimd.collective_compute(
                "AllGather", mybir.AluOpType.bypass,
                ins=[h2sh.opt()], outs=[h2full.opt()],
                replica_groups=[list(range(NC))])
            for hh in range(2):
                nc.sync.dma_start(
                    out=_ap(h2tab, [[R2 * P, NT // 2], [R2, P], [1, W2C]],
                            extra_offset=hh * (NT // 2) * P * R2),
                    in_=_ap(h2full, [[W2C * P, NT // 2], [W2C, P], [1, W2C]],
                            extra_offset=hh * (NT // 2) * P * W2C))
            nc.sync.dma_start(
                out=_ap(sco2, [[R2, P], [P * R2, NS], [1, 1]]),
                in_=sdst2_all[:])

            # ---------- layer 2 edge pass (same slot structure)
            for bb in range(nb_run):
                bat = plan["batches"][bb]
                nbch = bat["nbchunks"]
                ni = nbch * P
                itf = ix.tile([P, nbcmax * 16], i16, tag="it")
                it = itf[:, :nbch * 16]
                nc.sync.dma_start(
                    out=it, in_=gidx[:, bat["gbase"] * 16:
                                     (bat["gbase"] + nbch) * 16])

                G2 = gg.tile([P, nbcmax * W2C], bf16, tag="G2")
                coff = 0
                for b in range(NBANK):
                    nch = bat["banks"][b]
                    if nch == 0:
                        continue
                    dgather(nc.gpsimd, G2, coff,
                            _ap(h2tab, [[R2, bank_rows[b]], [1, W2C]],
                                extra_offset=b * BANK * R2),
                            itf, coff * 8, nch, W2C, R2)
                    coff += nch
                D2f = sb.tile([P, nbcmax * 2], bf16, tag="D2")
                D2 = D2f[:, :nbch * 2]
                dgather(nc.gpsimd, D2f, 0,
                        _ap(sco2, [[R2, LROW], [1, 2]]),
                        itf, nbch * 8, nbch, 2, R2)

                Stf = sb.tile([P, nbcmax], bf16, tag="St")
                St = Stf[:, :nbch]
                nc.vector.tensor_tensor(
                    out=St,
                    in0=_ap(G2, [G2[:].ap[0], [W2C, nbch]], extra_offset=32),
                    in1=_ap(D2f, [D2f[:].ap[0], [2, nbch]]), op=ALU.add)
                St2f = sb.tile([P, nbcmax], bf16, tag="St2")
                St2 = St2f[:, :nbch]
                nc.vector.scalar_tensor_tensor(
                    out=St2, in0=St, scalar=NEG_SLOPE, in1=St,
                    op0=ALU.mult, op1=ALU.max)
                nc.scalar.activation(
                    _ap(G2, [G2[:].ap[0], [W2C, nbch]], extra_offset=32),
                    St2, AF.Exp)
                nc.vector.tensor_tensor(
                    out=_ap(G2, [G2[:].ap[0], [W2C, nbch], [1, 32]]),
                    in0=_ap(G2, [G2[:].ap[0], [W2C, nbch], [1, 32]]),
                    in1=_ap(G2, [G2[:].ap[0], [W2C, nbch], [0, 32]],
                            extra_offset=32),
                    op=ALU.mult)

                ob = sb.tile([P, B * OUT_DIM], f32, tag="ob")
                for t in range(B):
                    s = bb * B + t
                    ncs = int(cs[s])
                    M = mp.tile([P, csmax * P], bf16, tag="M")
                    nc.vector.tensor_tensor(
                        out=_ap(M, [M[:].ap[0], [P, ncs], [1, P]]),
                        in0=_ap(dl_t, [dl_t[:].ap[0], [1, ncs], [0, P]],
                                extra_offset=int(scol[s])),
                        in1=_ap(iota_f, [iota_f[:].ap[0], [0, ncs], [1, P]]),
                        op=ALU.is_equal)
                    agg2 = psagg.tile([P, 33], f32, tag="agg")
                    for j in range(ncs):
                        gp0 = bat["posmap"][(s, j)] * W2C
                        nc.tensor.matmul(out=agg2[:], lhsT=M[:, j * P:(j + 1) * P],
                                         rhs=G2[:, gp0:gp0 + 33],
                                         start=(j == 0), stop=(j == ncs - 1))
                    den2 = sb.tile([P, 1], f32, tag="den2")
                    nc.vector.tensor_scalar(out=den2[:], in0=agg2[:, 32:33],
                                            scalar1=1e-30, scalar2=None,
                                            op0=ALU.max)
                    r2 = sb.tile([P, 1], f32, tag="r2")
                    nc.vector.reciprocal(out=r2[:], in_=den2[:])
                    nc.vector.tensor_scalar(
                        out=ob[:, t * OUT_DIM:(t + 1) * OUT_DIM],
                        in0=agg2[:, 0:32], scalar1=r2[:, 0:1], scalar2=None,
                        op0=ALU.mult)
                nc.sync.dma_start(
                    out=_ap(out2, [[OUT_DIM, P], [P * OUT_DIM, B], [1, OUT_DIM]],
                            extra_offset=bb * B * P * OUT_DIM),
                    in_=ob[:])

    nc.compile()
    return nc


def _install_ntff_shim():
    import contextlib
    import ctypes
    import types

    mod = types.ModuleType("antenv.axon_hooks")

    def _hook_factory(so_path="/opt/axon/libaxon_pjrt.so"):
        try:
            lib = ctypes.CDLL(so_path)
        except OSError:
            return None
        if not hasattr(lib, "axon_start_nrt_profile"):
            return None
        lib.axon_start_nrt_profile.argtypes = [
            ctypes.POINTER(ctypes.c_int64), ctypes.c_size_t]
        lib.axon_start_nrt_profile.restype = ctypes.c_int64
        lib.axon_stop_nrt_profile.argtypes = [ctypes.c_char_p]
        lib.axon_stop_nrt_profile.restype = ctypes.c_int64

        @contextlib.contextmanager
        def _hook(output_dir, device_ids):
            import jax
            jax.devices()
            if device_ids:
                ids = (ctypes.c_int64 * len(device_ids))(*device_ids)
                rc = lib.axon_start_nrt_profile(ids, len(device_ids))
            else:
                rc = lib.axon_start_nrt_profile(None, 0)
            if rc != 0:
                raise RuntimeError(f"axon_start_nrt_profile rc={rc}")
            try:
                yield
            finally:
                n = lib.axon_stop_nrt_profile(str(output_dir).encode())
                if n < 0:
                    raise RuntimeError(f"axon_stop_nrt_profile rc={n}")

        return _hook

    mod.get_axon_ntff_profile_hook = _hook_factory
    mod.set_axon_ntff_profile_hook = lambda h: None
    sys.modules["antenv.axon_hooks"] = mod
    from concourse import bass_utils as bu
    bu.upload_artifacts = lambda tmpdir: tmpdir


def _wrap16(flat):
    """[n] int16 -> [128, n/16] wrapped-per-16, replicated to 8 cores."""
    n = flat.shape[0]
    assert n % 16 == 0
    arr = flat.reshape(n // 16, 16).T          # [16, n/16]
    return np.tile(arr, (8, 1)).astype(np.int16)


def _prep_inputs(x, edge_index, W1, a_src1, a_dst1, W2, a_src2, a_dst2):
    import ml_dtypes

    x = np.asarray(x, np.float32)
    ei = np.asarray(edge_index)
    src = np.concatenate([ei[0], np.arange(N, dtype=np.int64)]).astype(np.int64)
    dst = np.concatenate([ei[1], np.arange(N, dtype=np.int64)]).astype(np.int64)

    tile_of = dst >> 7
    bank_of = src >> 15
    core_of = tile_of // NS
    s_of = tile_of % NS

    # counts per (core, s, bank) and shared chunk counts nbc [NS, 4]
    key = (core_of * NS + s_of) * NBANK + bank_of
    cnt = np.bincount(key, minlength=NC * NS * NBANK).reshape(NC, NS, NBANK)
    nbc = np.ceil(cnt.max(axis=0) / P).astype(np.int64)       # [NS, 4]
    plan = _plan(nbc)
    cs, scol, tcs = plan["cs"], plan["scol"], plan["tcs"]

    # rank of each edge within its (core, s, bank) block
    order = np.argsort(key, kind="stable")
    ksort = key[order]
    blk_starts = np.zeros(NC * NS * NBANK, np.int64)
    np.cumsum(np.bincount(key, minlength=NC * NS * NBANK)[:-1], out=blk_starts[1:])
    rank = np.arange(len(order), dtype=np.int64) - blk_starts[ksort]
    # per edge (in sorted order): slot = (p, chunk k within (s,bank) block)
    e_core = core_of[order]
    e_s = s_of[order]
    e_b = bank_of[order]
    e_p = rank % P
    e_k = rank // P
    e_src = src[order]
    e_dstloc = (dst[order] & 127).astype(np.float32)
    e_dstlocal = (dst[order] - core_of[order] * SHARD).astype(np.int64)

    # gpos of (s, b, k): from plan batches
    gpos_of = np.zeros((NS, NBANK, int(nbc.max()) + 1), np.int64)
    for bb in range(NB):
        bat = plan["batches"][bb]
        for s in range(bb * B, bb * B + B):
            j = 0
            for b in range(NBANK):
                for k in range(int(nbc[s][b])):
                    gpos_of[s, b, k] = bat["gbase"] + bat["posmap"][(s, j)]
                    j += 1
    # tile-major col of (s, b, k): scol[s] + (chunks of banks < b) + k
    joff = np.zeros((NS, NBANK), np.int64)
    for s in range(NS):
        acc = 0
        for b in range(NBANK):
            joff[s, b] = acc
            acc += int(nbc[s][b])

    e_gpos = gpos_of[e_s, e_b, e_k]
    e_tcol = scol[e_s] + joff[e_s, e_b] + e_k

    # per-core arrays
    in_maps = []
    W1 = np.asarray(W1, np.float32)
    a_src1 = np.asarray(a_src1, np.float32)
    a_dst1 = np.asarray(a_dst1, np.float32)
    W1h = W1.reshape(IN_DIM, HEADS, HID)
    W1s = np.einsum("khc,hc->kh", W1h, a_src1)
    W1d = np.einsum("khc,hc->kh", W1h, a_dst1)
    W1cat = np.concatenate([W1, W1s, W1d], axis=1)            # [128, 136]
    W2 = np.asarray(W2, np.float32)
    w2s = W2 @ np.asarray(a_src2, np.float32)[0]
    w2d = W2 @ np.asarray(a_dst2, np.float32)[0]
    W2cat = np.concatenate(
        [W2, w2s[:, None], w2d[:, None], np.zeros((IN_DIM, 2), np.float32)],
        axis=1)                                               # [128, 36]

    xT = np.zeros((P, NPAD), np.float32)
    xT[:, :N] = x.T
    xT = xT.astype(ml_dtypes.bfloat16)
    W1cat_b = W1cat.astype(ml_dtypes.bfloat16)
    W2cat_b = W2cat.astype(ml_dtypes.bfloat16)

    node_g = np.arange(NPAD, dtype=np.int64)
    for c in range(NC):
        sel = e_core == c
        gsrc_vals = np.zeros((tcs, P), np.int16)
        gdst_vals = np.zeros((tcs, P), np.int16)
        dl_vals = np.full((tcs, P), -1.0, np.float32)
        gsrc_vals[e_gpos[sel], e_p[sel]] = (e_src[sel] - e_b[sel] * BANK).astype(
            np.int16)
        gdst_vals[e_gpos[sel], e_p[sel]] = e_dstlocal[sel].astype(np.int16)
        dl_vals[e_tcol[sel], e_p[sel]] = e_dstloc[sel]

        # pack gidx: per batch: [gsrc banks-blocks | gdst whole-batch block]
        blocks = []
        for bb in range(NB):
            bat = plan["batches"][bb]
            g0, nbch = bat["gbase"], bat["nbchunks"]
            blocks.append(_wrap16(gsrc_vals[g0:g0 + nbch].reshape(-1)))
            blocks.append(_wrap16(gdst_vals[g0:g0 + nbch].reshape(-1)))
        gidx_c = np.concatenate(blocks, axis=1)               # [128, tcs*16]

        mine = (node_g >= c * SHARD) & (node_g < (c + 1) * SHARD)
        scat = np.where(mine, node_g - c * SHARD,
                        SHARD + (node_g & 127)).astype(np.int16)
        scatidx_c = _wrap16(scat)

        edloc_c = np.ascontiguousarray(dl_vals.T)             # [128, tcs]
        in_maps.append({
            "xT": xT, "W1cat": W1cat_b, "W2cat": W2cat_b,
            "gidx": gidx_c, "scatidx": scatidx_c, "edloc": edloc_c,
        })
    return in_maps, nbc


def kernel(**inputs):
    global _RUNNER, _RUNNER_KEY
    from concourse.bass_utils import run_bass_kernel_spmd

    trace = os.environ.get("GAT_TRACE") == "1"
    if trace:
        _install_ntff_shim()

    in_maps, nbc = _prep_inputs(
        inputs["x"], inputs["edge_index"], inputs["W1"], inputs["a_src1"],
        inputs["a_dst1"], inputs["W2"], inputs["a_src2"], inputs["a_dst2"])

    key = nbc.tobytes()
    if _RUNNER is None or _RUNNER_KEY != key:
        if os.environ.get("GAT_SMOKE") == "1":
            _RUNNER = _build_program(nbc, nb_run=2, p0_groups=2)
        else:
            _RUNNER = _build_program(nbc)
        _RUNNER_KEY = key
    nc = _RUNNER

    kw = {}
    if trace:
        import tempfile
        kw = dict(trace=True, tmpdir=tempfile.mkdtemp())
    res = run_bass_kernel_spmd(nc, in_maps, list(range(NC)), **kw)
    if trace and res.exec_time_ns is not None:
        print(f"HW exec time: {res.exec_time_ns} ns")
        kernel.last_exec_time_ns = res.exec_time_ns

    full = np.concatenate([res.results[c]["out2"] for c in range(NC)], axis=0)
    out = full[:N] + np.asarray(inputs["b2"], np.float32)[None, :]
    return out.astype(np.float32)


# revision 11
# speedup vs baseline: 1.6913x; 1.6913x over previous
"""Two-layer GAT on 8 Trainium2 NeuronCores (Bass/Tile, SPMD).

Sharding: dst nodes split into 784 tiles of 128; core c owns the 98
contiguous tiles = nodes [c*12544, (c+1)*12544).  Edges (incl.
self-loops) are grouped by dst tile, padded to a uniform 19 chunks of
128 edges per tile, so the device program is input-independent.

Per chunk: indirect-DMA gather of source rows, is_equal one-hot
(edge -> local dst), PE transpose of the one-hot to expand the tile's
contiguous dst scores to edges, then one PE matmul segment-reduces the
ex-scaled features plus the softmax denominators into PSUM.  Segment
max is algebraically dropped (scores are O(1), exp cannot overflow;
softmax is shift-invariant so results match to rounding).  Between
layers the per-shard [h2 | s_src2 | s_dst2] tables are AllGathered.
"""
import os
import sys

sys.path.insert(0, "/opt/trn_rl_repo")

import numpy as np

N = 100000
IN_DIM = 128
HID = 32
HEADS = 4
OUT_DIM = 32
NEG_SLOPE = 0.2

NC = 8
P = 128
NPAD = 100352          # 784 tiles of 128
SHARD = NPAD // NC     # 12544
NS = SHARD // P        # 98 dst tiles per core
CS = 19                # chunks of 128 edges per dst tile
TC = NS * CS           # 1862 chunks per core
NT = NPAD // P         # 784
W1C = 136              # h1(128) | ssrc1(4) | sdst1(4)
W2C = 36               # h2(32) | ssrc2(1) | sdst2(1) | pad(2)

_RUNNER = None


def _ap(t, ap_dims, extra_offset=0):
    import concourse.bass as bass
    base = t[:]
    return bass.AP(base.tensor, base.offset + extra_offset, ap_dims)


def _build_program(ns_run=NS, p0_groups=None):
    from concourse import bass, mybir, bacc
    import concourse.tile as tile
    from concourse.masks import make_identity

    f32 = mybir.dt.float32
    bf16 = mybir.dt.bfloat16
    i32 = mybir.dt.int32
    AF = mybir.ActivationFunctionType
    ALU = mybir.AluOpType

    nc = bacc.Bacc("TRN2", target_bir_lowering=False, debug=False, num_devices=NC)

    xT = nc.dram_tensor("xT", [P, NPAD], bf16, kind="ExternalInput")
    W1cat = nc.dram_tensor("W1cat", [P, W1C], bf16, kind="ExternalInput")
    W2cat = nc.dram_tensor("W2cat", [P, W2C], f32, kind="ExternalInput")
    esrc = nc.dram_tensor("esrc", [P, TC], i32, kind="ExternalInput")
    edloc = nc.dram_tensor("edloc", [P, TC], f32, kind="ExternalInput")
    dtids = nc.dram_tensor("dtids", [P, NS], i32, kind="ExternalInput")
    out2 = nc.dram_tensor("out2", [SHARD, OUT_DIM], f32, kind="ExternalOutput")
    dbg = os.environ.get("GAT_DEBUG") == "1"
    if dbg:
        h1dbg = nc.dram_tensor("h1dbg", [1024, W1C], bf16, kind="ExternalOutput")
        h2dbg = nc.dram_tensor("h2dbg", [SHARD, W2C], bf16, kind="ExternalOutput")
        sd_d = nc.dram_tensor("sd_d", [P, CS * 4], f32, kind="ExternalOutput")
        s_d = nc.dram_tensor("s_d", [P, CS * 4], f32, kind="ExternalOutput")
        ex_d = nc.dram_tensor("ex_d", [P, CS * 4], f32, kind="ExternalOutput")
        m_d = nc.dram_tensor("m_d", [P, CS * P], f32, kind="ExternalOutput")
        g_d = nc.dram_tensor("g_d", [P, CS * W1C], f32, kind="ExternalOutput")
        agg_d = nc.dram_tensor("agg_d", [P, 132], f32, kind="ExternalOutput")
        sdt_d = nc.dram_tensor("sdt_d", [P, W1C], f32, kind="ExternalOutput")
    h1ext = nc.dram_tensor("h1ext", [NPAD, W1C], bf16)

    with tile.TileContext(nc) as tc:
        with (
            tc.tile_pool(name="consts", bufs=1) as consts,
            tc.tile_pool(name="sb", bufs=4) as sb,
            tc.tile_pool(name="gx", bufs=3) as gx,
            tc.tile_pool(name="ps", bufs=2, space="PSUM") as ps,
            tc.tile_pool(name="pst", bufs=2, space="PSUM") as pst,
            tc.tile_pool(name="psagg", bufs=2, space="PSUM") as psagg,
            tc.tile_pool(name="dram", bufs=1, space="DRAM") as dram,
        ):
            ident = consts.tile([P, P], f32)
            make_identity(nc, ident[:])
            identb = consts.tile([P, P], bf16)
            nc.vector.tensor_copy(out=identb[:], in_=ident[:])
            iota_i = consts.tile([P, P], i32)
            nc.gpsimd.iota(iota_i[:], pattern=[[1, P]], base=0, channel_multiplier=0)
            iota_f = consts.tile([P, P], f32)
            nc.vector.tensor_copy(out=iota_f[:], in_=iota_i[:])
            w1_t = consts.tile([P, W1C], bf16)
            nc.sync.dma_start(out=w1_t[:], in_=W1cat[:])
            w2_t = consts.tile([P, W2C], f32)
            nc.sync.dma_start(out=w2_t[:], in_=W2cat[:])
            dt_t = consts.tile([P, NS], i32)
            nc.sync.dma_start(out=dt_t[:], in_=dtids[:])

            # ---------- phase 0: h1ext = [x@W1 | x@W1s | x@W1d], all nodes
            GRP = 8
            _ng = NT // GRP if p0_groups is None else p0_groups
            for g in range(_ng):
                xg = gx.tile([P, P * GRP], bf16, tag="xg")
                nc.sync.dma_start(out=xg[:], in_=xT[:, g * P * GRP:(g + 1) * P * GRP])
                for t in range(GRP):
                    p0 = ps.tile([P, W1C], f32, tag="p0")
                    nc.tensor.matmul(out=p0[:], lhsT=xg[:, t * P:(t + 1) * P],
                                     rhs=w1_t[:], start=True, stop=True)
                    s0 = sb.tile([P, W1C], bf16, tag="s0")
                    nc.scalar.copy(out=s0[:], in_=p0[:])
                    nc.sync.dma_start(
                        out=h1ext[(g * GRP + t) * P:(g * GRP + t + 1) * P, :],
                        in_=s0[:])

            h2sh = dram.tile([SHARD, W2C], bf16)
            h2full = dram.tile([NPAD, W2C], bf16)

            # ---------- layer 1 edge pass over own dst tiles
            for s in range(ns_run):
                c0 = s * CS
                # this slot's node rows (for sdst1, cols 132:136)
                sdt = sb.tile([P, W1C], bf16, tag="sdt")
                nc.gpsimd.indirect_dma_start(
                    out=sdt[:], out_offset=None, in_=h1ext[:],
                    in_offset=bass.IndirectOffsetOnAxis(ap=dt_t[:, s:s + 1], axis=0))
                dl = sb.tile([P, CS], f32, tag="dl")
                nc.sync.dma_start(out=dl[:], in_=edloc[:, c0:c0 + CS])
                es = sb.tile([P, CS], i32, tag="es")
                nc.sync.dma_start(out=es[:], in_=esrc[:, c0:c0 + CS])

                G = sb.tile([P, CS * W1C], bf16, tag="G")
                for j in range(CS):
                    nc.gpsimd.indirect_dma_start(
                        out=G[:, j * W1C:(j + 1) * W1C], out_offset=None,
                        in_=h1ext[:],
                        in_offset=bass.IndirectOffsetOnAxis(ap=es[:, j:j + 1], axis=0))

                # one-hot for all chunks: M[p, j*128+d] = (dl[p,j] == d)
                M = sb.tile([P, CS * P], bf16, tag="M")
                nc.vector.tensor_tensor(
                    out=_ap(M, [M[:].ap[0], [P, CS], [1, P]]),
                    in0=_ap(dl, [dl[:].ap[0], [1, CS], [0, P]]),
                    in1=_ap(iota_f, [iota_f[:].ap[0], [0, CS], [1, P]]),
                    op=ALU.is_equal)

                # per-edge sdst: SD[:, 4j:4j+4] = (M_j)^T.T-free expand
                SD = pst.tile([P, CS * 4], f32, tag="SD")
                for j in range(CS):
                    pT = pst.tile([P, P], bf16, tag="pT")
                    nc.tensor.transpose(out=pT[:], in_=M[:, j * P:(j + 1) * P],
                                        identity=identb[:])
                    mt = sb.tile([P, P], bf16, tag="mt")
                    nc.vector.tensor_copy(out=mt[:], in_=pT[:])
                    nc.tensor.matmul(out=SD[:, j * 4:(j + 1) * 4], lhsT=mt[:],
                                     rhs=sdt[:, 132:136], start=True, stop=True)

                # scores -> ex, written back into G's cols 128:132 per block
                SDb = sb.tile([P, CS * 4], bf16, tag="SDb")
                nc.vector.tensor_copy(out=SDb[:], in_=SD[:])
                S = sb.tile([P, CS * 4], bf16, tag="S")
                nc.vector.tensor_tensor(
                    out=S[:],
                    in0=_ap(G, [G[:].ap[0], [W1C, CS], [1, 4]], extra_offset=128),
                    in1=SDb[:], op=ALU.add)
                Sm = sb.tile([P, CS * 4], bf16, tag="Sm")
                nc.vector.tensor_scalar(out=Sm[:], in0=S[:], scalar1=NEG_SLOPE,
                                        scalar2=None, op0=ALU.mult)
                nc.vector.tensor_tensor(out=S[:], in0=S[:], in1=Sm[:], op=ALU.max)
                EX = sb.tile([P, CS * 4], bf16, tag="EX")
                nc.scalar.activation(EX[:], S[:], AF.Exp)
                nc.vector.tensor_copy(
                    out=_ap(G, [G[:].ap[0], [W1C, CS], [1, 4]], extra_offset=128),
                    in_=EX[:])
                # scale features by per-(edge, head) ex
                nc.vector.tensor_tensor(
                    out=_ap(G, [G[:].ap[0], [W1C, CS], [32, 4], [1, 32]]),
                    in0=_ap(G, [G[:].ap[0], [W1C, CS], [32, 4], [1, 32]]),
                    in1=_ap(G, [G[:].ap[0], [W1C, CS], [1, 4], [0, 32]],
                            extra_offset=128),
                    op=ALU.mult)

                agg = psagg.tile([P, 132], f32, tag="agg")
                for j in range(CS):
                    nc.tensor.matmul(out=agg[:], lhsT=M[:, j * P:(j + 1) * P],
                                     rhs=G[:, j * W1C:j * W1C + 132],
                                     start=(j == 0), stop=(j == CS - 1))

                # epilogue: divide, elu, h2 = h @ W2cat, store shard row block
                if dbg and s == 0:
                    sdcp = sb.tile([P, CS * 4], f32, tag="sdcp")
                    nc.vector.tensor_copy(out=sdcp[:], in_=SD[:])
                    nc.sync.dma_start(out=sd_d[:], in_=sdcp[:])
                    nc.sync.dma_start(out=s_d[:], in_=S[:])
                    nc.sync.dma_start(out=ex_d[:], in_=EX[:])
                    nc.sync.dma_start(out=m_d[:], in_=M[:])
                    nc.sync.dma_start(out=g_d[:], in_=G[:])
                    nc.sync.dma_start(out=sdt_d[:], in_=sdt[:])
                    agcp = sb.tile([P, 132], f32, tag="agcp")
                    nc.vector.tensor_copy(out=agcp[:], in_=agg[:])
                    nc.sync.dma_start(out=agg_d[:], in_=agcp[:])
                den = sb.tile([P, 4], f32, tag="den")
                nc.vector.tensor_scalar(out=den[:], in0=agg[:, 128:132],
                                        scalar1=1e-30, scalar2=None, op0=ALU.max)
                rden = sb.tile([P, 4], f32, tag="rden")
                nc.vector.reciprocal(out=rden[:], in_=den[:])
                h_t = sb.tile([P, P], f32, tag="h_t")
                nc.vector.tensor_tensor(
                    out=_ap(h_t, [h_t[:].ap[0], [32, 4], [1, 32]]),
                    in0=_ap(agg, [agg[:].ap[0], [32, 4], [1, 32]]),
                    in1=_ap(rden, [rden[:].ap[0], [1, 4], [0, 32]]),
                    op=ALU.mult)
                # elu(x) = max(x,0) + exp(min(x,0)) - 1
                neg = sb.tile([P, P], f32, tag="neg")
                nc.vector.tensor_scalar(out=neg[:], in0=h_t[:], scalar1=0.0,
                                        scalar2=None, op0=ALU.min)
                eneg = sb.tile([P, P], f32, tag="eneg")
                nc.scalar.activation(eneg[:], neg[:], AF.Exp)
                nc.vector.tensor_scalar(out=h_t[:], in0=h_t[:], scalar1=0.0,
                                        scalar2=None, op0=ALU.max)
                nc.vector.tensor_tensor(out=h_t[:], in0=h_t[:], in1=eneg[:],
                                        op=ALU.add)
                nc.vector.tensor_scalar(out=h_t[:], in0=h_t[:], scalar1=-1.0,
                                        scalar2=None, op0=ALU.add)
                hT = pst.tile([P, P], f32, tag="pT")
                nc.tensor.transpose(out=hT[:], in_=h_t[:], identity=ident[:])
                hTs = sb.tile([P, P], f32, tag="hTs")
                nc.vector.tensor_copy(out=hTs[:], in_=hT[:])
                h2p = ps.tile([P, W2C], f32, tag="p0")
                nc.tensor.matmul(out=h2p[:], lhsT=hTs[:], rhs=w2_t[:],
                                 start=True, stop=True)
                h2s = sb.tile([P, W2C], bf16, tag="h2s")
                nc.scalar.copy(out=h2s[:], in_=h2p[:])
                nc.sync.dma_start(out=h2sh[s * P:(s + 1) * P, :], in_=h2s[:])

            if dbg:
                for bb in range(8):
                    dtt = sb.tile([P, W1C], bf16, tag="dbg1")
                    nc.sync.dma_start(out=dtt[:], in_=h1ext[bb * P:(bb + 1) * P, :])
                    nc.sync.dma_start(out=h1dbg[bb * P:(bb + 1) * P, :], in_=dtt[:])
                for bb in range(NS):
                    dt2 = sb.tile([P, W2C], bf16, tag="dbg2")
                    nc.sync.dma_start(out=dt2[:], in_=h2sh[bb * P:(bb + 1) * P, :])
                    nc.sync.dma_start(out=h2dbg[bb * P:(bb + 1) * P, :], in_=dt2[:])

            # ---------- AllGather shard tables
            nc.gpsimd.collective_compute(
                "AllGather", mybir.AluOpType.bypass,
                ins=[h2sh.opt()], outs=[h2full.opt()],
                replica_groups=[list(range(NC))])

            # ---------- layer 2 edge pass (same chunk structure)
            for s in range(ns_run):
                c0 = s * CS
                sdt2 = sb.tile([P, W2C], bf16, tag="sdt2")
                nc.gpsimd.indirect_dma_start(
                    out=sdt2[:], out_offset=None, in_=h2full[:],
                    in_offset=bass.IndirectOffsetOnAxis(ap=dt_t[:, s:s + 1], axis=0))
                dl = sb.tile([P, CS], f32, tag="dl")
                nc.sync.dma_start(out=dl[:], in_=edloc[:, c0:c0 + CS])
                es = sb.tile([P, CS], i32, tag="es")
                nc.sync.dma_start(out=es[:], in_=esrc[:, c0:c0 + CS])

                G2 = sb.tile([P, CS * W2C], bf16, tag="G2")
                for j in range(CS):
                    nc.gpsimd.indirect_dma_start(
                        out=G2[:, j * W2C:(j + 1) * W2C], out_offset=None,
                        in_=h2full[:],
                        in_offset=bass.IndirectOffsetOnAxis(ap=es[:, j:j + 1], axis=0))

                M = sb.tile([P, CS * P], bf16, tag="M")
                nc.vector.tensor_tensor(
                    out=_ap(M, [M[:].ap[0], [P, CS], [1, P]]),
                    in0=_ap(dl, [dl[:].ap[0], [1, CS], [0, P]]),
                    in1=_ap(iota_f, [iota_f[:].ap[0], [0, CS], [1, P]]),
                    op=ALU.is_equal)

                SD = pst.tile([P, CS], f32, tag="SD")
                for j in range(CS):
                    pT = pst.tile([P, P], bf16, tag="pT")
                    nc.tensor.transpose(out=pT[:], in_=M[:, j * P:(j + 1) * P],
                                        identity=identb[:])
                    mt = sb.tile([P, P], bf16, tag="mt")
                    nc.vector.tensor_copy(out=mt[:], in_=pT[:])
                    nc.tensor.matmul(out=SD[:, j:j + 1], lhsT=mt[:],
                                     rhs=sdt2[:, 33:34], start=True, stop=True)

                SDb2 = sb.tile([P, CS], bf16, tag="SDb")
                nc.vector.tensor_copy(out=SDb2[:], in_=SD[:])
                S = sb.tile([P, CS], bf16, tag="S2")
                nc.vector.tensor_tensor(
                    out=S[:],
                    in0=_ap(G2, [G2[:].ap[0], [W2C, CS], [1, 1]], extra_offset=32),
                    in1=SDb2[:], op=ALU.add)
                Sm2 = sb.tile([P, CS], bf16, tag="Sm")
                nc.vector.tensor_scalar(out=Sm2[:], in0=S[:], scalar1=NEG_SLOPE,
                                        scalar2=None, op0=ALU.mult)
                nc.vector.tensor_tensor(out=S[:], in0=S[:], in1=Sm2[:], op=ALU.max)
                EX2 = sb.tile([P, CS], bf16, tag="EX2")
                nc.scalar.activation(EX2[:], S[:], AF.Exp)
                nc.vector.tensor_copy(
                    out=_ap(G2, [G2[:].ap[0], [W2C, CS], [1, 1]], extra_offset=32),
                    in_=EX2[:])
                nc.vector.tensor_tensor(
                    out=_ap(G2, [G2[:].ap[0], [W2C, CS], [1, 32]]),
                    in0=_ap(G2, [G2[:].ap[0], [W2C, CS], [1, 32]]),
                    in1=_ap(G2, [G2[:].ap[0], [W2C, CS], [0, 32]],
                            extra_offset=32),
                    op=ALU.mult)

                agg2 = psagg.tile([P, 33], f32, tag="agg")
                for j in range(CS):
                    nc.tensor.matmul(out=agg2[:], lhsT=M[:, j * P:(j + 1) * P],
                                     rhs=G2[:, j * W2C:j * W2C + 33],
                                     start=(j == 0), stop=(j == CS - 1))

                den2 = sb.tile([P, 1], f32, tag="den2")
                nc.vector.tensor_scalar(out=den2[:], in0=agg2[:, 32:33],
                                        scalar1=1e-30, scalar2=None, op0=ALU.max)
                r2 = sb.tile([P, 1], f32, tag="r2")
                nc.vector.reciprocal(out=r2[:], in_=den2[:])
                o_t = sb.tile([P, OUT_DIM], f32, tag="o_t")
                nc.vector.tensor_scalar(out=o_t[:], in0=agg2[:, 0:32],
                                        scalar1=r2[:, 0:1], scalar2=None,
                                        op0=ALU.mult)
                nc.sync.dma_start(out=out2[s * P:(s + 1) * P, :], in_=o_t[:])

    nc.compile()
    return nc


def _install_ntff_shim():
    import contextlib
    import ctypes
    import types

    mod = types.ModuleType("antenv.axon_hooks")

    def _hook_factory(so_path="/opt/axon/libaxon_pjrt.so"):
        try:
            lib = ctypes.CDLL(so_path)
        except OSError:
            return None
        if not hasattr(lib, "axon_start_nrt_profile"):
            return None
        lib.axon_start_nrt_profile.argtypes = [
            ctypes.POINTER(ctypes.c_int64), ctypes.c_size_t]
        lib.axon_start_nrt_profile.restype = ctypes.c_int64
        lib.axon_stop_nrt_profile.argtypes = [ctypes.c_char_p]
        lib.axon_stop_nrt_profile.restype = ctypes.c_int64

        @contextlib.contextmanager
        def _hook(output_dir, device_ids):
            import jax
            jax.devices()
            if device_ids:
                ids = (ctypes.c_int64 * len(device_ids))(*device_ids)
                rc = lib.axon_start_nrt_profile(ids, len(device_ids))
            else:
                rc = lib.axon_start_nrt_profile(None, 0)
            if rc != 0:
                raise RuntimeError(f"axon_start_nrt_profile rc={rc}")
            try:
                yield
            finally:
                n = lib.axon_stop_nrt_profile(str(output_dir).encode())
                if n < 0:
                    raise RuntimeError(f"axon_stop_nrt_profile rc={n}")

        return _hook

    mod.get_axon_ntff_profile_hook = _hook_factory
    mod.set_axon_ntff_profile_hook = lambda h: None
    sys.modules["antenv.axon_hooks"] = mod
    from concourse import bass_utils as bu
    bu.upload_artifacts = lambda tmpdir: tmpdir


def _prep_inputs(x, edge_index, W1, a_src1, a_dst1, W2, a_src2, a_dst2):
    import ml_dtypes

    x = np.asarray(x, np.float32)
    ei = np.asarray(edge_index)
    src = np.concatenate([ei[0], np.arange(N, dtype=np.int64)]).astype(np.int64)
    dst = np.concatenate([ei[1], np.arange(N, dtype=np.int64)]).astype(np.int64)
    Etot = src.shape[0]

    tile_of = (dst >> 7).astype(np.int64)
    counts = np.bincount(tile_of, minlength=NT)
    if counts.max() > CS * P:
        raise ValueError(f"dst tile overflow: {counts.max()} > {CS * P}")
    order = np.argsort(tile_of, kind="stable")
    starts = np.zeros(NT, np.int64)
    np.cumsum(counts[:-1], out=starts[1:])
    tile_sorted = tile_of[order]
    pos = np.arange(Etot, dtype=np.int64) - starts[tile_sorted]

    src_pad = np.zeros((NT, CS * P), np.int32)
    dloc_pad = np.full((NT, CS * P), -1.0, np.float32)
    src_pad[tile_sorted, pos] = src[order].astype(np.int32)
    dloc_pad[tile_sorted, pos] = (dst[order] & 127).astype(np.float32)

    # weights
    W1 = np.asarray(W1, np.float32)                       # [128, 128]
    a_src1 = np.asarray(a_src1, np.float32)               # [4, 32]
    a_dst1 = np.asarray(a_dst1, np.float32)
    W1h = W1.reshape(IN_DIM, HEADS, HID)
    W1s = np.einsum("khc,hc->kh", W1h, a_src1)            # [128, 4]
    W1d = np.einsum("khc,hc->kh", W1h, a_dst1)
    W1cat = np.concatenate([W1, W1s, W1d], axis=1)        # [128, 136]

    W2 = np.asarray(W2, np.float32)                       # [128, 32]
    w2s = W2 @ np.asarray(a_src2, np.float32)[0]          # [128]
    w2d = W2 @ np.asarray(a_dst2, np.float32)[0]
    W2cat = np.concatenate(
        [W2, w2s[:, None], w2d[:, None], np.zeros((IN_DIM, 2), np.float32)],
        axis=1)                                           # [128, 36]

    xT = np.zeros((P, NPAD), np.float32)
    xT[:, :N] = x.T
    xT = xT.astype(ml_dtypes.bfloat16)
    W1cat_b = W1cat.astype(ml_dtypes.bfloat16)

    in_maps = []
    for c in range(NC):
        tiles = slice(c * NS, (c + 1) * NS)
        esrc_c = np.ascontiguousarray(
            src_pad[tiles].reshape(NS, CS, P).transpose(2, 0, 1).reshape(P, TC))
        edloc_c = np.ascontiguousarray(
            dloc_pad[tiles].reshape(NS, CS, P).transpose(2, 0, 1).reshape(P, TC))
        dtids_c = (c * SHARD + np.arange(NS)[None, :] * P
                   + np.arange(P)[:, None]).astype(np.int32)
        in_maps.append({
            "xT": xT, "W1cat": W1cat_b, "W2cat": W2cat,
            "esrc": esrc_c, "edloc": edloc_c, "dtids": dtids_c,
        })
    return in_maps


def kernel(**inputs):
    global _RUNNER
    from concourse.bass_utils import run_bass_kernel_spmd

    trace = os.environ.get("GAT_TRACE") == "1"
    if trace:
        _install_ntff_shim()

    if _RUNNER is None:
        if os.environ.get("GAT_SMOKE") == "1":
            _RUNNER = _build_program(ns_run=2, p0_groups=2)
        else:
            _RUNNER = _build_program()
    nc = _RUNNER

    in_maps = _prep_inputs(
        inputs["x"], inputs["edge_index"], inputs["W1"], inputs["a_src1"],
        inputs["a_dst1"], inputs["W2"], inputs["a_src2"], inputs["a_dst2"])

    kw = {}
    if trace:
        import tempfile
        kw = dict(trace=True, tmpdir=tempfile.mkdtemp())
    res = run_bass_kernel_spmd(nc, in_maps, list(range(NC)), **kw)
    if trace and res.exec_time_ns is not None:
        print(f"HW exec time: {res.exec_time_ns} ns")
        kernel.last_exec_time_ns = res.exec_time_ns

    full = np.concatenate([res.results[c]["out2"] for c in range(NC)], axis=0)
    out = full[:N] + np.asarray(inputs["b2"], np.float32)[None, :]
    return out.astype(np.float32)

